# revision 15
# baseline (speedup 1.0000x reference)
"""Multi-head causal self-attention (B=2, T=4096, C=768, H=12, D=64) on 8 trn2 cores.

Sharding: core c -> batch b = c//4, head group g = c%4 (3 heads each).
Each core computes qkv projection for its heads, causal attention, and a
row-parallel partial of the output projection; the host sums the 4 partials
per batch and adds b_out.

Device algorithm (per core), bf16 matmuls with f32 PSUM accumulation:
  qT/kT[h] [64, T] = (w_qk_h | b)^T @ (x | 1)^T      (bias via augmented row)
  vsb [tk, nkv, h, 65] = v chunks + ones column       (denominator trick)
  Attention processes kv-chunk PAIRS (2m, 2m+1) per 512-wide query tile I:
    S^T planes -> st [128, 2, 512] f32 psum   (2 matmuls, diag-trimmed)
    pt [128, 2, 512] bf16 = exp(st / 8)       (ONE ACT instr per pair)
    diag planes: memset invalid cols 0, triangle window *= 0/1 mask (DVE 4x)
    ot [65, 512] += vaug_j^T @ pt plane       (row 64 = softmax denominators)
  aT[h] = ot[0:64] * broadcast(1/ot[64]); y partial = aT^T @ w_out -> DRAM.

Scheduling: projection / normalize / epilogue work is deferred into a FIFO
and dripped one unit per attention pair so PE/DVE work fills the gaps under
the ACT-bound exp stream instead of serializing between blocks.
"""

import sys

sys.path.insert(0, "/opt/trn_rl_repo")

from contextlib import ExitStack

import numpy as np

import concourse.bass as bass
import concourse.bacc as bacc
import concourse.mybir as mybir
from concourse import tile
from concourse.bass_utils import run_bass_kernel_spmd

B, T, C, H, D = 2, 4096, 768, 12, 64
HPC = 3  # heads per core
NCORES = 8
P = 128
NKV = T // P  # 32 kv chunks of 128
NI = T // 512  # 8 query super-tiles of 512
KC = C // P  # 6 full contraction chunks (+1 bias row)

BF16 = mybir.dt.bfloat16
F32 = mybir.dt.float32
NPBF16 = np.dtype(mybir.dt.np(BF16))

TRACE = False  # set by test.py to collect HW exec time
LAST = None  # last BassKernelResults

_prog = None
_last_in_maps = None


def bench(n=5):
    """Re-run the compiled kernel n times; returns per-run wall seconds."""
    import time

    times = []
    for _ in range(n):
        t0 = time.time()
        run_bass_kernel_spmd(_prog, _last_in_maps, list(range(NCORES)))
        times.append(time.time() - t0)
    return times


def _build():
    nc = bacc.Bacc(
        "TRN2",
        target_bir_lowering=False,
        debug=False,
        enable_asserts=False,
        num_devices=NCORES,
    )
    xt = nc.declare_dram_parameter("xt", [C + 1, T], BF16, False)
    wqk = nc.declare_dram_parameter("wqk", [C + 1, 2 * D * HPC], BF16, False)
    wv = nc.declare_dram_parameter("wv", [C + 1, D * HPC], BF16, False)
    wout = nc.declare_dram_parameter("wout", [D * HPC, C], BF16, False)
    msk = nc.declare_dram_parameter("msk", [P, P], BF16, False)
    y = nc.declare_dram_parameter("y", [T, C], F32, True)

    with ExitStack() as ctx:
        tc = ctx.enter_context(tile.TileContext(nc))
        cp = ctx.enter_context(tc.tile_pool(name="const", bufs=1))
        pb = ctx.enter_context(tc.tile_pool(name="pbuf", bufs=3))
        pyo = ctx.enter_context(tc.tile_pool(name="pyout", bufs=4))
        pr = ctx.enter_context(tc.tile_pool(name="pr", bufs=3))
        ps = ctx.enter_context(tc.tile_pool(name="psum", bufs=2, space="PSUM"))

        xt_sb = [
            cp.tile([P, T], BF16, tag=f"xt{p}", name=f"xt_sb{p}") for p in range(KC)
        ]
        xt_sb.append(cp.tile([1, T], BF16, tag="xt6", name="xt_sb6"))
        wqk_sb = [
            cp.tile([P, 2 * D * HPC], BF16, tag=f"wqk{p}", name=f"wqk_sb{p}")
            for p in range(KC)
        ]
        wqk_sb.append(cp.tile([1, 2 * D * HPC], BF16, tag="wqk6", name="wqk_sb6"))
        wv_sb = [
            cp.tile([P, D * HPC], BF16, tag=f"wv{p}", name=f"wv_sb{p}")
            for p in range(KC)
        ]
        wv_sb.append(cp.tile([1, D * HPC], BF16, tag="wv6", name="wv_sb6"))
        wo_sb = [
            cp.tile([D, C], BF16, tag=f"wo{h}", name=f"wo_sb{h}") for h in range(HPC)
        ]
        msk_sb = cp.tile([P, P], BF16, tag="msk", name="msk_sb")
        qT = [cp.tile([D, T], BF16, tag=f"qT{h}", name=f"qT{h}") for h in range(HPC)]
        kT = [cp.tile([D, T], BF16, tag=f"kT{h}", name=f"kT{h}") for h in range(HPC)]
        vsb = cp.tile([P, NKV, HPC, D + 1], BF16, tag="v", name="vsb")
        aT = [cp.tile([D, T], BF16, tag=f"aT{h}", name=f"aT{h}") for h in range(HPC)]
        ones64 = cp.tile([1, D], F32, tag="ones64", name="ones64")

        # ---- input loads ----
        qs = [nc.sync, nc.scalar]
        for p in range(KC):
            qs[p % 2].dma_start(xt_sb[p][:], xt[p * P : (p + 1) * P, :])
            qs[(p + 1) % 2].dma_start(wqk_sb[p][:], wqk[p * P : (p + 1) * P, :])
            qs[(p + 1) % 2].dma_start(wv_sb[p][:], wv[p * P : (p + 1) * P, :])
        nc.scalar.dma_start(xt_sb[KC][:], xt[C : C + 1, :])
        nc.scalar.dma_start(wqk_sb[KC][:], wqk[C : C + 1, :])
        nc.scalar.dma_start(wv_sb[KC][:], wv[C : C + 1, :])
        for h in range(HPC):
            nc.scalar.dma_start(wo_sb[h][:], wout[h * D : (h + 1) * D, :])
        nc.scalar.dma_start(msk_sb[:], msk[:])
        nc.vector.memset(ones64[:], 1.0)
        nc.vector.memset(vsb[:, :, :, D : D + 1], 1.0)

        # ---- deferred-work FIFO: dripped into the attention pair loop ----
        deferred = []

        def drip(k=1):
            for _ in range(k):
                if deferred:
                    deferred.pop(0)()

        def qk_unit(h, n):
            # projection of one 512-wide tile of qT/kT for head h
            def run():
                t = ps.tile([P, 512], F32, tag="y", name="qk_ps")
                tq = slice(512 * n, 512 * (n + 1))
                for p in range(KC + 1):
                    nc.tensor.matmul(
                        t[:],
                        wqk_sb[p][:, P * h : P * (h + 1)],
                        xt_sb[p][:, tq],
                        start=(p == 0),
                        stop=(p == KC),
                    )
                nc.vector.tensor_copy(qT[h][:, tq], t[0:D, :])
                nc.vector.tensor_copy(kT[h][:, tq], t[D : 2 * D, :])

            return run

        def v_unit(n):
            # projection of v chunk n (128 tokens, all heads)
            def run():
                t = ps.tile([P, 512], F32, tag="y", name="v_ps")
                tv = t[:, 0 : HPC * D]
                for p in range(KC + 1):
                    nc.tensor.matmul(
                        tv,
                        xt_sb[p][:, P * n : P * (n + 1)],
                        wv_sb[p][:],
                        start=(p == 0),
                        stop=(p == KC),
                    )
                nc.vector.tensor_copy(
                    vsb[:, n, :, 0:D],
                    tv.rearrange("p (h d) -> p h d", h=HPC),
                )

            return run

        def norm_a_unit(h, I, ot, box):
            # aT[h][:, I] = ot rows 0:64; r = 1 / ot row 64
            def run():
                sl = slice(512 * I, 512 * (I + 1))
                r_sb = pr.tile([1, 512], F32, tag="r", name="r_sb")
                nc.vector.reciprocal(r_sb[:], ot[D : D + 1, :])
                nc.vector.tensor_copy(aT[h][:, sl], ot[0:D, :])
                box.append(r_sb)

            return run

        def norm_b_unit(h, I, box):
            # aT[h][:, I] *= broadcast(r); popped a drip after norm_a so the
            # rb matmul doesn't stall PE on the reciprocal
            def run():
                sl = slice(512 * I, 512 * (I + 1))
                rb = ps.tile([D, 512], F32, tag="o", name="rb_ps")
                nc.tensor.matmul(rb[:], ones64[:], box[0][:], start=True, stop=True)
                nc.vector.tensor_mul(aT[h][:, sl], aT[h][:, sl], rb[:])

            return run

        def epi_unit(tck):
            # out-projection partial for one 128-row block of y
            def run():
                ya = ps.tile([P, 512], F32, tag="y", name="ya_ps")
                yb = ps.tile([P, 256], F32, tag="o", name="yb_ps")
                for h in range(HPC):
                    lhsT = aT[h][:, P * tck : P * (tck + 1)]
                    nc.tensor.matmul(
                        ya[:], lhsT, wo_sb[h][:, 0:512], start=(h == 0), stop=(h == 2)
                    )
                    nc.tensor.matmul(
                        yb[:],
                        lhsT,
                        wo_sb[h][:, 512:768],
                        start=(h == 0),
                        stop=(h == 2),
                    )
                ysb = pyo.tile([P, C], F32, tag="ysb", name="ysb")
                nc.vector.tensor_copy(ysb[:, 0:512], ya[:])
                nc.vector.tensor_copy(ysb[:, 512:768], yb[:])
                nc.sync.dma_start(y[P * tck : P * (tck + 1), :], ysb[:])

            return run

        # ---- prologue projections: qk tiles I'=0,1 all heads; v chunks 0..7
        for h in range(HPC):
            for n in range(2):
                qk_unit(h, n)()
        for n in range(8):
            v_unit(n)()

        # remaining projections through the drip FIFO, ordered by deadline:
        # attention tile I needs qk tile I (all h) and v chunks <= 4I+3;
        # attention tile I provides 3*(2I+2) drip slots before tile I+1.
        for I in range(2, NI):
            for h in range(HPC):
                deferred.append(qk_unit(h, I))
            for n in range(4 * I, 4 * I + 4):
                deferred.append(v_unit(n))

        # ---- attention ----
        def attn_block(h, I):
            ot = ps.tile([D + 1, 512], F32, tag="o", name="o_ps")
            jmax = 4 * I + 3
            npair = 2 * I + 2
            pts = {}

            def emit_st(m):
                # S^T planes for pair m, then one exp instruction
                st = ps.tile([P, 2, 512], F32, tag="s2", name="s_ps")
                for pl in range(2):
                    j = 2 * m + pl
                    v = j - 4 * I
                    c0 = 128 * v if v > 0 else 0
                    nc.tensor.matmul(
                        st[:, pl, c0:512],
                        kT[h][:, P * j : P * (j + 1)],
                        qT[h][:, 512 * I + c0 : 512 * (I + 1)],
                        start=True,
                        stop=True,
                    )
                pt = pb.tile([P, 2, 512], BF16, tag="p", name="p_sb")
                nc.scalar.activation(
                    pt[:], st[:], mybir.ActivationFunctionType.Exp, scale=0.125
                )
                pts[m] = pt

            def emit_pv(m):
                pt = pts.pop(m)
                for pl in range(2):
                    j = 2 * m + pl
                    v = j - 4 * I
                    if v >= 0:
                        if v > 0:
                            nc.vector.memset(pt[:, pl, 0 : 128 * v], 0.0)
                        nc.vector.tensor_mul(
                            pt[:, pl, 128 * v : 128 * (v + 1)],
                            pt[:, pl, 128 * v : 128 * (v + 1)],
                            msk_sb[:],
                        )
                    # PV trimmed to valid cols except the last chunk, which
                    # covers full width so the psum group stop is clean
                    # (invalid pt cols are zeroed above).
                    c0 = 128 * v if 0 < v < 3 else 0
                    nc.tensor.matmul(
                        ot[:, c0:512],
                        vsb[:, j, h, :],
                        pt[:, pl, c0:512],
                        start=(j == 0),
                        stop=(j == jmax),
                    )

            # 1-stage software pipeline: S^T/exp of pair m+1 is emitted
            # before PV of pair m so PE never waits on the exp in flight.
            emit_st(0)
            for m in range(1, npair):
                emit_st(m)
                emit_pv(m - 1)
                drip(2 if I >= NI - 2 else 1)
            emit_pv(npair - 1)
            drip()
            box = []
            deferred.insert(0, norm_b_unit(h, I, box))
            deferred.insert(0, norm_a_unit(h, I, ot, box))

        for I in range(NI):
            for h in range(HPC):
                attn_block(h, I)
            for t in range(4):
                deferred.append(epi_unit(4 * I + t))

        drip(len(deferred))

    nc.compile()
    return nc


def _masks():
    """Triangle window mask [128, 128]: m[p, c] = 0 if c < p else 1."""
    p = np.arange(P)[:, None]
    c = np.arange(P)[None, :]
    return (c >= p).astype(np.float32).astype(NPBF16)


def _inputs_for_core(c, x, w_qkv, b_qkv, w_out, masks):
    b, g = divmod(c, 4)
    h0 = HPC * g
    xt = np.empty((C + 1, T), np.float32)
    xt[:C] = x[b].T
    xt[C] = 1.0
    wfull = np.concatenate([w_qkv, b_qkv[None, :]], axis=0)  # [C+1, 3C]
    qk_cols = []
    for h in range(h0, h0 + HPC):
        qk_cols.extend(range(D * h, D * h + D))
        qk_cols.extend(range(C + D * h, C + D * h + D))
    return {
        "xt": xt.astype(NPBF16),
        "wqk": np.ascontiguousarray(wfull[:, qk_cols]).astype(NPBF16),
        "wv": np.ascontiguousarray(
            wfull[:, 2 * C + D * h0 : 2 * C + D * (h0 + HPC)]
        ).astype(NPBF16),
        "wout": np.ascontiguousarray(w_out[D * h0 : D * (h0 + HPC), :]).astype(NPBF16),
        "msk": masks,
    }


def kernel(x, w_qkv, b_qkv, w_out, b_out):
    global _prog, LAST
    x = np.asarray(x, np.float32)
    w_qkv = np.asarray(w_qkv, np.float32)
    b_qkv = np.asarray(b_qkv, np.float32)
    w_out = np.asarray(w_out, np.float32)
    b_out = np.asarray(b_out, np.float32)
    if _prog is None:
        _prog = _build()
    masks = _masks()
    in_maps = [
        _inputs_for_core(c, x, w_qkv, b_qkv, w_out, masks) for c in range(NCORES)
    ]
    global _last_in_maps
    _last_in_maps = in_maps
    LAST = run_bass_kernel_spmd(_prog, in_maps, list(range(NCORES)))
    out = np.zeros((B, T, C), np.float32)
    for c in range(NCORES):
        out[c // 4] += np.asarray(LAST.results[c]["y"], np.float32)
    out += b_out[None, None, :]
    return out
